# revision 6
# baseline (speedup 1.0000x reference)
"""Chamfer loss kernel for Trainium2 (8 NeuronCores).

Problem: pred [16384, 3], target [16384, 3] fp32.
  dist[i, j] = ||pred_i - target_j||
  out = (mean_i min_j dist + mean_j min_i dist) / 2

Strategy:
  - Shard pred rows across the 8 cores (2048 rows each); every core sees the
    full target set.
  - On each core compute the transposed squared-distance matrix
        S^T[t, p] = ||pred_p - target_t||^2
    via a single K=5 augmented matmul:
        lhsT rows = [-2*tx, -2*ty, -2*tz, |t|^2, 1]   (stationary, targets)
        rhs  rows = [ px,    py,   pz,   1,  |p|^2]   (moving, preds)
    so the TensorEngine emits squared distances directly into PSUM.
    Since K=5 only uses 5 of the 128 PE array rows, the 4 pred-chunks of a
    target block are packed into 4 concurrent row-group matmuls
    (tile_position=(32*i, 0)) writing 4 different PSUM banks.
  - VectorE per 128-target block:
      * tensor_scalar (min, +inf) reading PSUM fp32 with a free-dim
        min-accumulate -> exact backward-direction partial min over this
        core's 2048 preds; its elementwise output doubles as the SBUF
        staging copy (cast to bf16 unless CHAMFER_EXACT=1).
      * tensor_tensor min into a [128, 2048] running accumulator (bf16 runs
        in the DVE 2x mode) -> forward-direction partial.
  - Tail: 16 PE transposes + free-dim min reduces fold the forward
    accumulator across partitions.
  - Host: sqrt/clamp/means + 8-way elementwise min for the backward
    direction (min over squared distances commutes with the monotone
    sqrt(max(.,0))), so reducing over squared distances is exact.
"""

import os
import sys

if "/opt/trn_rl_repo" not in sys.path:
    sys.path.insert(0, "/opt/trn_rl_repo")

from contextlib import ExitStack

import numpy as np

import concourse.bass as bass
import concourse.mybir as mybir
import concourse.tile as tile
from concourse import bacc
from concourse.bass_utils import run_bass_kernel_spmd
from concourse.masks import make_identity

F32 = mybir.dt.float32
BF16 = mybir.dt.bfloat16
BIG = 3.0e38

N_CORES = 8
N = 16384  # pred rows (global)
M = 16384  # target rows
R = N // N_CORES  # pred rows per core = 2048
TB = M // 128  # target blocks = 128
PC = R // 512  # pred chunks per core = 4 (one per PE row-group)

EXACT = bool(int(os.environ.get("CHAMFER_EXACT", "0")))

_cache = {}


def _build():
    s_dt = F32 if EXACT else BF16

    nc = bacc.Bacc("TRN2", target_bir_lowering=False, debug=False, num_devices=N_CORES)

    t_aug_d = nc.dram_tensor("t_aug", [5, M], F32, kind="ExternalInput")
    p_aug_d = nc.dram_tensor("p_aug", [5, R], F32, kind="ExternalInput")
    o_col_d = nc.dram_tensor("o_col", [128, TB], F32, kind="ExternalOutput")
    o_row_d = nc.dram_tensor("o_row", [128, R // 128], F32, kind="ExternalOutput")

    with tile.TileContext(nc) as tc:
        with ExitStack() as ctx:
            const = ctx.enter_context(tc.tile_pool(name="const", bufs=1))
            spool = ctx.enter_context(tc.tile_pool(name="spool", bufs=3))
            pspool = ctx.enter_context(tc.tile_pool(name="pspool", bufs=2, space="PSUM"))

            # weights/rhs replicated at partition bases 0/32/64/96 for the
            # 4 concurrent row-group matmuls
            t_w = const.tile([128, M], F32)
            p_r = const.tile([128, R], F32)
            for i in range(4):
                nc.sync.dma_start(t_w[32 * i : 32 * i + 5, :], t_aug_d.ap())
                nc.sync.dma_start(p_r[32 * i : 32 * i + 5, :], p_aug_d.ap())

            ident = const.tile([128, 128], F32)
            make_identity(nc, ident[:])

            rowacc = const.tile([128, R], s_dt)
            nc.vector.memset(rowacc[:], BIG)
            rowacc32 = const.tile([128, R], F32)
            colmin = const.tile([128, TB], F32)
            orow = const.tile([128, R // 128], F32)

            if EXACT:
                for tb in range(TB):
                    s_ps = pspool.tile([128, R], F32, tag="s_ps")
                    for pc in range(PC):
                        nc.tensor.matmul(
                            s_ps[:, pc * 512 : (pc + 1) * 512],
                            t_w[32 * pc : 32 * pc + 5, tb * 128 : (tb + 1) * 128],
                            p_r[32 * pc : 32 * pc + 5, pc * 512 : (pc + 1) * 512],
                            start=True,
                            stop=True,
                            tile_position=(32 * pc, 0),
                        )
                    s_sb = spool.tile([128, R], s_dt, tag="s_sb")
                    # backward partial: exact fp32 min over this core's preds
                    # for each target; elementwise out is the SBUF staging copy
                    nc.vector.tensor_scalar(
                        out=s_sb[:],
                        in0=s_ps[:],
                        scalar1=BIG,
                        scalar2=None,
                        op0=mybir.AluOpType.min,
                        op1=mybir.AluOpType.min,
                        accum_out=colmin[:, tb : tb + 1],
                    )
                    # forward running min across target blocks
                    nc.vector.tensor_tensor(
                        rowacc[:], rowacc[:], s_sb[:], op=mybir.AluOpType.min
                    )
            else:
                G = 4  # target blocks per colmin-tree batch
                for tbg in range(TB // G):
                    sgrp = spool.tile([128, G * R], s_dt, tag="s_sb")
                    for g in range(G):
                        tb = tbg * G + g
                        s_ps = pspool.tile([128, R], F32, tag="s_ps")
                        for pc in range(PC):
                            nc.tensor.matmul(
                                s_ps[:, pc * 512 : (pc + 1) * 512],
                                t_w[32 * pc : 32 * pc + 5, tb * 128 : (tb + 1) * 128],
                                p_r[32 * pc : 32 * pc + 5, pc * 512 : (pc + 1) * 512],
                                start=True,
                                stop=True,
                                tile_position=(32 * pc, 0),
                            )
                        # ScalarE does the PSUM->SBUF bf16 cast
                        nc.scalar.copy(sgrp[:, g * R : (g + 1) * R], s_ps[:])
                        # forward running min across target blocks (bf16 2x)
                        nc.vector.tensor_tensor(
                            rowacc[:],
                            rowacc[:],
                            sgrp[:, g * R : (g + 1) * R],
                            op=mybir.AluOpType.min,
                        )
                    # batched colmin tree over the G blocks (bf16 2x TTs + one
                    # small 1x reduce; the accumulate-reduce opcode has no
                    # fast uop so a TT tree is cheaper)
                    sv = sgrp[:].rearrange("p (g n) -> p g n", g=G)
                    tr = spool.tile([128, G * (R // 2)], s_dt, tag="tree")
                    tv = tr[:].rearrange("p (g n) -> p g n", g=G)
                    nc.vector.tensor_tensor(
                        tv[:, :, :], sv[:, :, 0 : R // 2], sv[:, :, R // 2 : R],
                        op=mybir.AluOpType.min,
                    )
                    w = R // 4
                    while w >= 128:
                        nc.vector.tensor_tensor(
                            tv[:, :, 0:w], tv[:, :, 0:w], tv[:, :, w : 2 * w],
                            op=mybir.AluOpType.min,
                        )
                        w //= 2
                    nc.vector.tensor_reduce(
                        out=colmin[:, tbg * G : (tbg + 1) * G],
                        in_=tv[:, :, 0:128],
                        axis=mybir.AxisListType.X,
                        op=mybir.AluOpType.min,
                    )

            # fold rowacc across partitions: transpose 128x128 blocks (fp32),
            # then min-reduce the free dim
            if EXACT:
                rowacc32 = rowacc
            else:
                nc.vector.tensor_copy(rowacc32[:], rowacc[:])
            for t in range(R // 128):
                tr_ps = pspool.tile([128, 128], F32, tag="s_ps")
                nc.tensor.transpose(
                    tr_ps[:], rowacc32[:, t * 128 : (t + 1) * 128], ident[:]
                )
                nc.vector.tensor_reduce(
                    out=orow[:, t : t + 1],
                    in_=tr_ps[:],
                    axis=mybir.AxisListType.X,
                    op=mybir.AluOpType.min,
                )

            nc.sync.dma_start(o_col_d.ap(), colmin[:])
            nc.sync.dma_start(o_row_d.ap(), orow[:])

    nc.compile()
    return nc


def _prepare_inputs(pred, target):
    pred = np.ascontiguousarray(np.asarray(pred, dtype=np.float32))
    target = np.ascontiguousarray(np.asarray(target, dtype=np.float32))
    t2 = (target * target).sum(axis=1)
    p2 = (pred * pred).sum(axis=1)
    ones_m = np.ones(M, dtype=np.float32)
    t_aug = np.stack(
        [-2.0 * target[:, 0], -2.0 * target[:, 1], -2.0 * target[:, 2], t2, ones_m],
        axis=0,
    ).astype(np.float32)
    t_aug = np.ascontiguousarray(t_aug)
    in_maps = []
    for k in range(N_CORES):
        sl = slice(k * R, (k + 1) * R)
        p = pred[sl]
        p_aug = np.stack(
            [p[:, 0], p[:, 1], p[:, 2], np.ones(R, dtype=np.float32), p2[sl]], axis=0
        ).astype(np.float32)
        in_maps.append({"t_aug": t_aug, "p_aug": np.ascontiguousarray(p_aug)})
    return in_maps


def _run(pred, target, trace=False):
    if "nc" not in _cache:
        _cache["nc"] = _build()
    nc = _cache["nc"]
    in_maps = _prepare_inputs(pred, target)
    res = run_bass_kernel_spmd(nc, in_maps, core_ids=list(range(N_CORES)), trace=trace)

    rowmins = []
    colparts = []
    for k in range(N_CORES):
        out = res.results[k]
        # o_row[q, t] -> pred local index t*128+q
        rowmins.append(out["o_row"].T.reshape(-1))
        # o_col[p, tb] -> target index tb*128+p
        colparts.append(out["o_col"].T.reshape(-1))
    rowmin_sq = np.concatenate(rowmins)  # [16384] squared forward mins
    colmin_sq = np.min(np.stack(colparts, axis=0), axis=0)  # [16384]

    fwd = np.sqrt(np.maximum(rowmin_sq, 0.0)).mean()
    bwd = np.sqrt(np.maximum(colmin_sq, 0.0)).mean()
    value = np.float32((fwd + bwd) / 2.0)
    return np.asarray(value, dtype=np.float32), res


def kernel(pred, target):
    out, _ = _run(pred, target, trace=False)
    return out


# revision 16
# speedup vs baseline: 1.2594x; 1.2594x over previous
"""Chamfer loss kernel for Trainium2 (8 NeuronCores).

Problem: pred [16384, 3], target [16384, 3] fp32.
  dist[i, j] = ||pred_i - target_j||
  out = (mean_i min_j dist + mean_j min_i dist) / 2

Strategy:
  - Shard pred rows across the 8 cores (2048 rows each); every core sees the
    full target set.
  - On each core compute the transposed squared-distance matrix
        S^T[t, p] = ||pred_p - target_t||^2
    via a single K=5 augmented matmul:
        lhsT rows = [-2*tx, -2*ty, -2*tz, |t|^2, 1]   (stationary, targets)
        rhs  rows = [ px,    py,   pz,   1,  |p|^2]   (moving, preds)
    so the TensorEngine emits squared distances directly into PSUM.
    Since K=5 only uses 5 of the 128 PE array rows, the 4 pred-chunks of a
    target block are packed into 4 concurrent row-group matmuls
    (tile_position=(32*i, 0)) writing 4 different PSUM banks.
  - VectorE per 128-target block:
      * tensor_scalar (min, +inf) reading PSUM fp32 with a free-dim
        min-accumulate -> exact backward-direction partial min over this
        core's 2048 preds; its elementwise output doubles as the SBUF
        staging copy (cast to bf16 unless CHAMFER_EXACT=1).
      * tensor_tensor min into a [128, 2048] running accumulator (bf16 runs
        in the DVE 2x mode) -> forward-direction partial.
  - Tail: 16 PE transposes + free-dim min reduces fold the forward
    accumulator across partitions.
  - Host: sqrt/clamp/means + 8-way elementwise min for the backward
    direction (min over squared distances commutes with the monotone
    sqrt(max(.,0))), so reducing over squared distances is exact.
"""

import os
import sys

if "/opt/trn_rl_repo" not in sys.path:
    sys.path.insert(0, "/opt/trn_rl_repo")

from contextlib import ExitStack

import numpy as np

import concourse.bass as bass
import concourse.mybir as mybir
import concourse.tile as tile
from concourse import bacc
from concourse.bass_utils import run_bass_kernel_spmd
from concourse.masks import make_identity

F32 = mybir.dt.float32
BF16 = mybir.dt.bfloat16
BIG = 3.0e38

N_CORES = 8
N = 16384  # pred rows (global)
M = 16384  # target rows
R = N // N_CORES  # pred rows per core = 2048
TB = M // 128  # target blocks = 128
PC = R // 512  # pred chunks per core = 4 (one per PE row-group)

EXACT = bool(int(os.environ.get("CHAMFER_EXACT", "0")))

_cache = {}


def _build():
    s_dt = F32 if EXACT else BF16

    nc = bacc.Bacc("TRN2", target_bir_lowering=False, debug=False, num_devices=N_CORES)

    t_aug_d = nc.dram_tensor("t_aug", [5, M], F32, kind="ExternalInput")
    p_aug_d = nc.dram_tensor("p_aug", [5, R], F32, kind="ExternalInput")
    o_col_d = nc.dram_tensor("o_col", [128, TB], F32, kind="ExternalOutput")
    o_row_d = nc.dram_tensor("o_row", [128, R // 128], F32, kind="ExternalOutput")

    with tile.TileContext(nc) as tc:
        with ExitStack() as ctx:
            const = ctx.enter_context(tc.tile_pool(name="const", bufs=1))
            spool = ctx.enter_context(tc.tile_pool(name="spool", bufs=3))
            pspool = ctx.enter_context(tc.tile_pool(name="pspool", bufs=2, space="PSUM"))

            # weights/rhs replicated at partition bases 0/32/64/96 for the
            # 4 concurrent row-group matmuls
            # spread the replica loads over all three DMA issuers (sync/act
            # HWDGE + gpsimd SWDGE); the rhs replicas and the first weight
            # columns load first in small chunks so the matmul pipeline can
            # start while the rest of the weights stream in behind it
            t_w = const.tile([128, M], F32)
            p_r = const.tile([128, R], F32)
            dma_engines = [nc.sync, nc.scalar, nc.gpsimd]
            idx = 0
            for i in range(4):
                dma_engines[idx % 3].dma_start(
                    p_r[32 * i : 32 * i + 5, :], p_aug_d.ap()
                )
                idx += 1
            # graduated chunks: small ones first so the first matmuls unblock
            # quickly, then larger ones streaming behind compute
            col_chunks = [512] * 4 + [2048] * 7
            c0 = 0
            for ch in col_chunks:
                cols = slice(c0, c0 + ch)
                c0 += ch
                for i in range(4):
                    dma_engines[idx % 3].dma_start(
                        t_w[32 * i : 32 * i + 5, cols], t_aug_d.ap()[:, cols]
                    )
                    idx += 1

            ident = const.tile([128, 128], F32)
            make_identity(nc, ident[:])

            rowacc = const.tile([128, R], s_dt)
            nc.vector.memset(rowacc[:], BIG)
            rowacc32 = const.tile([128, R], F32)
            colmin = const.tile([128, TB], F32)
            orow = const.tile([128, R // 128], F32)

            if EXACT:
                for tb in range(TB):
                    s_ps = pspool.tile([128, R], F32, tag="s_ps")
                    for pc in range(PC):
                        nc.tensor.matmul(
                            s_ps[:, pc * 512 : (pc + 1) * 512],
                            t_w[32 * pc : 32 * pc + 5, tb * 128 : (tb + 1) * 128],
                            p_r[32 * pc : 32 * pc + 5, pc * 512 : (pc + 1) * 512],
                            start=True,
                            stop=True,
                            tile_position=(32 * pc, 0),
                        )
                    s_sb = spool.tile([128, R], s_dt, tag="s_sb")
                    # backward partial: exact fp32 min over this core's preds
                    # for each target; elementwise out is the SBUF staging copy
                    nc.vector.tensor_scalar(
                        out=s_sb[:],
                        in0=s_ps[:],
                        scalar1=BIG,
                        scalar2=None,
                        op0=mybir.AluOpType.min,
                        op1=mybir.AluOpType.min,
                        accum_out=colmin[:, tb : tb + 1],
                    )
                    # forward running min across target blocks
                    nc.vector.tensor_tensor(
                        rowacc[:], rowacc[:], s_sb[:], op=mybir.AluOpType.min
                    )
            else:
                G = 4  # target blocks per colmin-tree batch
                for tbg in range(TB // G):
                    sgrp = spool.tile([128, G * R], s_dt, tag="s_sb")
                    for g in range(G):
                        tb = tbg * G + g
                        s_ps = pspool.tile([128, R], F32, tag="s_ps")
                        for pc in range(PC):
                            nc.tensor.matmul(
                                s_ps[:, pc * 512 : (pc + 1) * 512],
                                t_w[32 * pc : 32 * pc + 5, tb * 128 : (tb + 1) * 128],
                                p_r[32 * pc : 32 * pc + 5, pc * 512 : (pc + 1) * 512],
                                start=True,
                                stop=True,
                                tile_position=(32 * pc, 0),
                            )
                        # ScalarE does the PSUM->SBUF bf16 cast
                        nc.scalar.copy(sgrp[:, g * R : (g + 1) * R], s_ps[:])
                        # forward running min across target blocks (bf16 2x)
                        nc.vector.tensor_tensor(
                            rowacc[:],
                            rowacc[:],
                            sgrp[:, g * R : (g + 1) * R],
                            op=mybir.AluOpType.min,
                        )
                    # batched colmin tree over the G blocks (bf16 2x TTs + one
                    # small 1x reduce; the accumulate-reduce opcode has no
                    # fast uop so a TT tree is cheaper)
                    sv = sgrp[:].rearrange("p (g n) -> p g n", g=G)
                    tr = spool.tile([128, G * (R // 2)], s_dt, tag="tree")
                    tv = tr[:].rearrange("p (g n) -> p g n", g=G)
                    nc.vector.tensor_tensor(
                        tv[:, :, :], sv[:, :, 0 : R // 2], sv[:, :, R // 2 : R],
                        op=mybir.AluOpType.min,
                    )
                    w = R // 4
                    while w >= 128:
                        nc.vector.tensor_tensor(
                            tv[:, :, 0:w], tv[:, :, 0:w], tv[:, :, w : 2 * w],
                            op=mybir.AluOpType.min,
                        )
                        w //= 2
                    nc.vector.tensor_reduce(
                        out=colmin[:, tbg * G : (tbg + 1) * G],
                        in_=tv[:, :, 0:128],
                        axis=mybir.AxisListType.X,
                        op=mybir.AluOpType.min,
                    )

            # fold rowacc across partitions: transpose 128x128 blocks (fp32),
            # then min-reduce the free dim
            if EXACT:
                rowacc32 = rowacc
            else:
                nc.vector.tensor_copy(rowacc32[:], rowacc[:])
            for t in range(R // 128):
                tr_ps = pspool.tile([128, 128], F32, tag="s_ps")
                nc.tensor.transpose(
                    tr_ps[:], rowacc32[:, t * 128 : (t + 1) * 128], ident[:]
                )
                nc.vector.tensor_reduce(
                    out=orow[:, t : t + 1],
                    in_=tr_ps[:],
                    axis=mybir.AxisListType.X,
                    op=mybir.AluOpType.min,
                )

            nc.sync.dma_start(o_col_d.ap(), colmin[:])
            nc.sync.dma_start(o_row_d.ap(), orow[:])

    nc.compile()
    return nc


def _prepare_inputs(pred, target):
    pred = np.ascontiguousarray(np.asarray(pred, dtype=np.float32))
    target = np.ascontiguousarray(np.asarray(target, dtype=np.float32))
    t2 = (target * target).sum(axis=1)
    p2 = (pred * pred).sum(axis=1)
    ones_m = np.ones(M, dtype=np.float32)
    t_aug = np.stack(
        [-2.0 * target[:, 0], -2.0 * target[:, 1], -2.0 * target[:, 2], t2, ones_m],
        axis=0,
    ).astype(np.float32)
    t_aug = np.ascontiguousarray(t_aug)
    in_maps = []
    for k in range(N_CORES):
        sl = slice(k * R, (k + 1) * R)
        p = pred[sl]
        p_aug = np.stack(
            [p[:, 0], p[:, 1], p[:, 2], np.ones(R, dtype=np.float32), p2[sl]], axis=0
        ).astype(np.float32)
        in_maps.append({"t_aug": t_aug, "p_aug": np.ascontiguousarray(p_aug)})
    return in_maps


def _run(pred, target, trace=False):
    if "nc" not in _cache:
        _cache["nc"] = _build()
    nc = _cache["nc"]
    in_maps = _prepare_inputs(pred, target)
    res = run_bass_kernel_spmd(nc, in_maps, core_ids=list(range(N_CORES)), trace=trace)

    rowmins = []
    colparts = []
    for k in range(N_CORES):
        out = res.results[k]
        # o_row[q, t] -> pred local index t*128+q
        rowmins.append(out["o_row"].T.reshape(-1))
        # o_col[p, tb] -> target index tb*128+p
        colparts.append(out["o_col"].T.reshape(-1))
    rowmin_sq = np.concatenate(rowmins)  # [16384] squared forward mins
    colmin_sq = np.min(np.stack(colparts, axis=0), axis=0)  # [16384]

    fwd = np.sqrt(np.maximum(rowmin_sq, 0.0)).mean()
    bwd = np.sqrt(np.maximum(colmin_sq, 0.0)).mean()
    value = np.float32((fwd + bwd) / 2.0)
    return np.asarray(value, dtype=np.float32), res


def kernel(pred, target):
    out, _ = _run(pred, target, trace=False)
    return out


# revision 17
# speedup vs baseline: 1.2616x; 1.0018x over previous
"""Chamfer loss kernel for Trainium2 (8 NeuronCores).

Problem: pred [16384, 3], target [16384, 3] fp32.
  dist[i, j] = ||pred_i - target_j||
  out = (mean_i min_j dist + mean_j min_i dist) / 2

Strategy:
  - Shard pred rows across the 8 cores (2048 rows each); every core sees the
    full target set.
  - On each core compute the transposed squared-distance matrix
        S^T[t, p] = ||pred_p - target_t||^2
    via a single K=5 augmented matmul:
        lhsT rows = [-2*tx, -2*ty, -2*tz, |t|^2, 1]   (stationary, targets)
        rhs  rows = [ px,    py,   pz,   1,  |p|^2]   (moving, preds)
    so the TensorEngine emits squared distances directly into PSUM.
    Since K=5 only uses 5 of the 128 PE array rows, the 4 pred-chunks of a
    target block are packed into 4 concurrent row-group matmuls
    (tile_position=(32*i, 0)) writing 4 different PSUM banks.
  - VectorE per 128-target block:
      * tensor_scalar (min, +inf) reading PSUM fp32 with a free-dim
        min-accumulate -> exact backward-direction partial min over this
        core's 2048 preds; its elementwise output doubles as the SBUF
        staging copy (cast to bf16 unless CHAMFER_EXACT=1).
      * tensor_tensor min into a [128, 2048] running accumulator (bf16 runs
        in the DVE 2x mode) -> forward-direction partial.
  - Tail: 16 PE transposes + free-dim min reduces fold the forward
    accumulator across partitions.
  - Host: sqrt/clamp/means + 8-way elementwise min for the backward
    direction (min over squared distances commutes with the monotone
    sqrt(max(.,0))), so reducing over squared distances is exact.
"""

import os
import sys

if "/opt/trn_rl_repo" not in sys.path:
    sys.path.insert(0, "/opt/trn_rl_repo")

from contextlib import ExitStack

import numpy as np

import concourse.bass as bass
import concourse.mybir as mybir
import concourse.tile as tile
from concourse import bacc
from concourse.bass_utils import run_bass_kernel_spmd
from concourse.masks import make_identity

F32 = mybir.dt.float32
BF16 = mybir.dt.bfloat16
BIG = 3.0e38

N_CORES = 8
N = 16384  # pred rows (global)
M = 16384  # target rows
R = N // N_CORES  # pred rows per core = 2048
TB = M // 128  # target blocks = 128
PC = R // 512  # pred chunks per core = 4 (one per PE row-group)

EXACT = bool(int(os.environ.get("CHAMFER_EXACT", "0")))

_cache = {}


def _build():
    s_dt = F32 if EXACT else BF16

    nc = bacc.Bacc("TRN2", target_bir_lowering=False, debug=False, num_devices=N_CORES)

    t_aug_d = nc.dram_tensor("t_aug", [5, M], F32, kind="ExternalInput")
    p_aug_d = nc.dram_tensor("p_aug", [5, R], F32, kind="ExternalInput")
    o_col_d = nc.dram_tensor("o_col", [128, TB], F32, kind="ExternalOutput")
    o_row_d = nc.dram_tensor("o_row", [128, R // 128], F32, kind="ExternalOutput")

    with tile.TileContext(nc) as tc:
        with ExitStack() as ctx:
            const = ctx.enter_context(tc.tile_pool(name="const", bufs=1))
            spool = ctx.enter_context(tc.tile_pool(name="spool", bufs=4))
            pspool = ctx.enter_context(tc.tile_pool(name="pspool", bufs=2, space="PSUM"))

            # weights/rhs replicated at partition bases 0/32/64/96 for the
            # 4 concurrent row-group matmuls
            # spread the replica loads over all three DMA issuers (sync/act
            # HWDGE + gpsimd SWDGE); the rhs replicas and the first weight
            # columns load first in small chunks so the matmul pipeline can
            # start while the rest of the weights stream in behind it
            t_w = const.tile([128, M], F32)
            p_r = const.tile([128, R], F32)
            dma_engines = [nc.sync, nc.scalar, nc.gpsimd]
            idx = 0
            for i in range(4):
                dma_engines[idx % 3].dma_start(
                    p_r[32 * i : 32 * i + 5, :], p_aug_d.ap()
                )
                idx += 1
            # graduated chunks: small ones first so the first matmuls unblock
            # quickly, then larger ones streaming behind compute
            col_chunks = [512] * 4 + [2048] * 7
            c0 = 0
            for ch in col_chunks:
                cols = slice(c0, c0 + ch)
                c0 += ch
                for i in range(4):
                    dma_engines[idx % 3].dma_start(
                        t_w[32 * i : 32 * i + 5, cols], t_aug_d.ap()[:, cols]
                    )
                    idx += 1

            ident = const.tile([128, 128], F32)
            make_identity(nc, ident[:])

            rowacc = const.tile([128, R], s_dt)
            nc.vector.memset(rowacc[:], BIG)
            rowacc32 = const.tile([128, R], F32)
            colmin = const.tile([128, TB], F32)
            orow = const.tile([128, R // 128], F32)

            if EXACT:
                for tb in range(TB):
                    s_ps = pspool.tile([128, R], F32, tag="s_ps")
                    for pc in range(PC):
                        nc.tensor.matmul(
                            s_ps[:, pc * 512 : (pc + 1) * 512],
                            t_w[32 * pc : 32 * pc + 5, tb * 128 : (tb + 1) * 128],
                            p_r[32 * pc : 32 * pc + 5, pc * 512 : (pc + 1) * 512],
                            start=True,
                            stop=True,
                            tile_position=(32 * pc, 0),
                        )
                    s_sb = spool.tile([128, R], s_dt, tag="s_sb")
                    # backward partial: exact fp32 min over this core's preds
                    # for each target; elementwise out is the SBUF staging copy
                    nc.vector.tensor_scalar(
                        out=s_sb[:],
                        in0=s_ps[:],
                        scalar1=BIG,
                        scalar2=None,
                        op0=mybir.AluOpType.min,
                        op1=mybir.AluOpType.min,
                        accum_out=colmin[:, tb : tb + 1],
                    )
                    # forward running min across target blocks
                    nc.vector.tensor_tensor(
                        rowacc[:], rowacc[:], s_sb[:], op=mybir.AluOpType.min
                    )
            else:
                G = 4  # target blocks per colmin-tree batch
                for tbg in range(TB // G):
                    sgrp = spool.tile([128, G * R], s_dt, tag="s_sb")
                    for g in range(G):
                        tb = tbg * G + g
                        s_ps = pspool.tile([128, R], F32, tag="s_ps")
                        for pc in range(PC):
                            nc.tensor.matmul(
                                s_ps[:, pc * 512 : (pc + 1) * 512],
                                t_w[32 * pc : 32 * pc + 5, tb * 128 : (tb + 1) * 128],
                                p_r[32 * pc : 32 * pc + 5, pc * 512 : (pc + 1) * 512],
                                start=True,
                                stop=True,
                                tile_position=(32 * pc, 0),
                            )
                        # ScalarE does the PSUM->SBUF bf16 cast
                        nc.scalar.copy(sgrp[:, g * R : (g + 1) * R], s_ps[:])
                        # forward running min across target blocks (bf16 2x)
                        nc.vector.tensor_tensor(
                            rowacc[:],
                            rowacc[:],
                            sgrp[:, g * R : (g + 1) * R],
                            op=mybir.AluOpType.min,
                        )
                    # batched colmin tree over the G blocks (bf16 2x TTs + one
                    # small 1x reduce; the accumulate-reduce opcode has no
                    # fast uop so a TT tree is cheaper)
                    sv = sgrp[:].rearrange("p (g n) -> p g n", g=G)
                    tr = spool.tile([128, G * (R // 2)], s_dt, tag="tree")
                    tv = tr[:].rearrange("p (g n) -> p g n", g=G)
                    nc.vector.tensor_tensor(
                        tv[:, :, :], sv[:, :, 0 : R // 2], sv[:, :, R // 2 : R],
                        op=mybir.AluOpType.min,
                    )
                    w = R // 4
                    while w >= 128:
                        nc.vector.tensor_tensor(
                            tv[:, :, 0:w], tv[:, :, 0:w], tv[:, :, w : 2 * w],
                            op=mybir.AluOpType.min,
                        )
                        w //= 2
                    nc.vector.tensor_reduce(
                        out=colmin[:, tbg * G : (tbg + 1) * G],
                        in_=tv[:, :, 0:128],
                        axis=mybir.AxisListType.X,
                        op=mybir.AluOpType.min,
                    )

            # fold rowacc across partitions: transpose 128x128 blocks (fp32),
            # then min-reduce the free dim
            if EXACT:
                rowacc32 = rowacc
            else:
                nc.vector.tensor_copy(rowacc32[:], rowacc[:])
            for t in range(R // 128):
                tr_ps = pspool.tile([128, 128], F32, tag="s_ps")
                nc.tensor.transpose(
                    tr_ps[:], rowacc32[:, t * 128 : (t + 1) * 128], ident[:]
                )
                nc.vector.tensor_reduce(
                    out=orow[:, t : t + 1],
                    in_=tr_ps[:],
                    axis=mybir.AxisListType.X,
                    op=mybir.AluOpType.min,
                )

            nc.sync.dma_start(o_col_d.ap(), colmin[:])
            nc.sync.dma_start(o_row_d.ap(), orow[:])

    nc.compile()
    return nc


def _prepare_inputs(pred, target):
    pred = np.ascontiguousarray(np.asarray(pred, dtype=np.float32))
    target = np.ascontiguousarray(np.asarray(target, dtype=np.float32))
    t2 = (target * target).sum(axis=1)
    p2 = (pred * pred).sum(axis=1)
    ones_m = np.ones(M, dtype=np.float32)
    t_aug = np.stack(
        [-2.0 * target[:, 0], -2.0 * target[:, 1], -2.0 * target[:, 2], t2, ones_m],
        axis=0,
    ).astype(np.float32)
    t_aug = np.ascontiguousarray(t_aug)
    in_maps = []
    for k in range(N_CORES):
        sl = slice(k * R, (k + 1) * R)
        p = pred[sl]
        p_aug = np.stack(
            [p[:, 0], p[:, 1], p[:, 2], np.ones(R, dtype=np.float32), p2[sl]], axis=0
        ).astype(np.float32)
        in_maps.append({"t_aug": t_aug, "p_aug": np.ascontiguousarray(p_aug)})
    return in_maps


def _run(pred, target, trace=False):
    if "nc" not in _cache:
        _cache["nc"] = _build()
    nc = _cache["nc"]
    in_maps = _prepare_inputs(pred, target)
    res = run_bass_kernel_spmd(nc, in_maps, core_ids=list(range(N_CORES)), trace=trace)

    rowmins = []
    colparts = []
    for k in range(N_CORES):
        out = res.results[k]
        # o_row[q, t] -> pred local index t*128+q
        rowmins.append(out["o_row"].T.reshape(-1))
        # o_col[p, tb] -> target index tb*128+p
        colparts.append(out["o_col"].T.reshape(-1))
    rowmin_sq = np.concatenate(rowmins)  # [16384] squared forward mins
    colmin_sq = np.min(np.stack(colparts, axis=0), axis=0)  # [16384]

    fwd = np.sqrt(np.maximum(rowmin_sq, 0.0)).mean()
    bwd = np.sqrt(np.maximum(colmin_sq, 0.0)).mean()
    value = np.float32((fwd + bwd) / 2.0)
    return np.asarray(value, dtype=np.float32), res


def kernel(pred, target):
    out, _ = _run(pred, target, trace=False)
    return out


# revision 22
# speedup vs baseline: 1.2830x; 1.0169x over previous
"""Chamfer loss kernel for Trainium2 (8 NeuronCores).

Problem: pred [16384, 3], target [16384, 3] fp32.
  dist[i, j] = ||pred_i - target_j||
  out = (mean_i min_j dist + mean_j min_i dist) / 2

Strategy:
  - Shard pred rows across the 8 cores (2048 rows each); every core sees the
    full target set.
  - On each core compute the transposed squared-distance matrix
        S^T[t, p] = ||pred_p - target_t||^2
    via a single K=5 augmented matmul:
        lhsT rows = [-2*tx, -2*ty, -2*tz, |t|^2, 1]   (stationary, targets)
        rhs  rows = [ px,    py,   pz,   1,  |p|^2]   (moving, preds)
    so the TensorEngine emits squared distances directly into PSUM.
    Since K=5 only uses 5 of the 128 PE array rows, the 4 pred-chunks of a
    target block are packed into 4 concurrent row-group matmuls
    (tile_position=(32*i, 0)) writing 4 different PSUM banks.
  - VectorE per 128-target block:
      * tensor_scalar (min, +inf) reading PSUM fp32 with a free-dim
        min-accumulate -> exact backward-direction partial min over this
        core's 2048 preds; its elementwise output doubles as the SBUF
        staging copy (cast to bf16 unless CHAMFER_EXACT=1).
      * tensor_tensor min into a [128, 2048] running accumulator (bf16 runs
        in the DVE 2x mode) -> forward-direction partial.
  - Tail: 16 PE transposes + free-dim min reduces fold the forward
    accumulator across partitions.
  - Host: sqrt/clamp/means + 8-way elementwise min for the backward
    direction (min over squared distances commutes with the monotone
    sqrt(max(.,0))), so reducing over squared distances is exact.
"""

import os
import sys

if "/opt/trn_rl_repo" not in sys.path:
    sys.path.insert(0, "/opt/trn_rl_repo")

from contextlib import ExitStack

import numpy as np

import concourse.bass as bass
import concourse.mybir as mybir
import concourse.tile as tile
from concourse import bacc
from concourse.bass_utils import run_bass_kernel_spmd
from concourse.masks import make_identity

F32 = mybir.dt.float32
BF16 = mybir.dt.bfloat16
BIG = 3.0e38

N_CORES = 8
N = 16384  # pred rows (global)
M = 16384  # target rows
R = N // N_CORES  # pred rows per core = 2048
TB = M // 128  # target blocks = 128
PC = R // 512  # pred chunks per core = 4 (one per PE row-group)

EXACT = bool(int(os.environ.get("CHAMFER_EXACT", "0")))

_cache = {}


def _build():
    s_dt = F32 if EXACT else BF16

    nc = bacc.Bacc("TRN2", target_bir_lowering=False, debug=False, num_devices=N_CORES)

    t_aug_d = nc.dram_tensor("t_aug", [5, M], F32, kind="ExternalInput")
    # rhs and first weight chunk come pre-replicated from the host as dense
    # 128-partition tensors -> one fast full-width DMA each instead of many
    # few-partition transfers on the critical path
    p_aug_d = nc.dram_tensor("p_aug", [128, R], F32, kind="ExternalInput")
    t_w0_d = nc.dram_tensor("t_w0", [128, 1024], F32, kind="ExternalInput")
    o_col_d = nc.dram_tensor("o_col", [128, TB], F32, kind="ExternalOutput")
    o_row_d = nc.dram_tensor("o_row", [128, R // 128], F32, kind="ExternalOutput")

    with tile.TileContext(nc) as tc:
        with ExitStack() as ctx:
            const = ctx.enter_context(tc.tile_pool(name="const", bufs=1))
            spool = ctx.enter_context(tc.tile_pool(name="spool", bufs=2))
            trpool = ctx.enter_context(tc.tile_pool(name="trpool", bufs=1))
            pspool = ctx.enter_context(tc.tile_pool(name="pspool", bufs=2, space="PSUM"))

            # weights replicated at partition bases 0/32/64/96 for the
            # 4 concurrent row-group matmuls; loads spread over all three DMA
            # issuers (sync/act HWDGE + gpsimd SWDGE) and column-chunked so
            # compute starts while the rest of the weights stream in behind
            t_w = const.tile([128, M], F32)
            p_r = const.tile([128, R], F32)
            nc.sync.dma_start(p_r[:], p_aug_d.ap())
            nc.scalar.dma_start(t_w[:, 0:1024], t_w0_d.ap())
            dma_engines = [nc.gpsimd, nc.sync, nc.scalar]
            idx = 0
            col_chunks = [512] * 2 + [2048] * 7
            c0 = 1024
            for ch in col_chunks:
                cols = slice(c0, c0 + ch)
                c0 += ch
                for i in range(4):
                    dma_engines[idx % 3].dma_start(
                        t_w[32 * i : 32 * i + 5, cols], t_aug_d.ap()[:, cols]
                    )
                    idx += 1

            ident = const.tile([128, 128], F32)
            make_identity(nc, ident[:])

            rowacc = const.tile([128, R], s_dt)
            nc.vector.memset(rowacc[:], BIG)
            rowacc32 = const.tile([128, R], F32)
            colmin = const.tile([128, TB], F32)
            orow = const.tile([128, R // 128], F32)

            if EXACT:
                for tb in range(TB):
                    s_ps = pspool.tile([128, R], F32, tag="s_ps")
                    for pc in range(PC):
                        nc.tensor.matmul(
                            s_ps[:, pc * 512 : (pc + 1) * 512],
                            t_w[32 * pc : 32 * pc + 5, tb * 128 : (tb + 1) * 128],
                            p_r[32 * pc : 32 * pc + 5, pc * 512 : (pc + 1) * 512],
                            start=True,
                            stop=True,
                            tile_position=(32 * pc, 0),
                        )
                    s_sb = spool.tile([128, R], s_dt, tag="s_sb")
                    # backward partial: exact fp32 min over this core's preds
                    # for each target; elementwise out is the SBUF staging copy
                    nc.vector.tensor_scalar(
                        out=s_sb[:],
                        in0=s_ps[:],
                        scalar1=BIG,
                        scalar2=None,
                        op0=mybir.AluOpType.min,
                        op1=mybir.AluOpType.min,
                        accum_out=colmin[:, tb : tb + 1],
                    )
                    # forward running min across target blocks
                    nc.vector.tensor_tensor(
                        rowacc[:], rowacc[:], s_sb[:], op=mybir.AluOpType.min
                    )
            else:
                G = 8  # target blocks per colmin-tree batch
                for tbg in range(TB // G):
                    sgrp = spool.tile([128, G * R], s_dt, tag="s_sb")
                    for g in range(G):
                        tb = tbg * G + g
                        s_ps = pspool.tile([128, R], F32, tag="s_ps")
                        for pc in range(PC):
                            nc.tensor.matmul(
                                s_ps[:, pc * 512 : (pc + 1) * 512],
                                t_w[32 * pc : 32 * pc + 5, tb * 128 : (tb + 1) * 128],
                                p_r[32 * pc : 32 * pc + 5, pc * 512 : (pc + 1) * 512],
                                start=True,
                                stop=True,
                                tile_position=(32 * pc, 0),
                            )
                        # ScalarE does the PSUM->SBUF bf16 cast
                        nc.scalar.copy(sgrp[:, g * R : (g + 1) * R], s_ps[:])
                        # forward running min across target blocks (bf16 2x)
                        nc.vector.tensor_tensor(
                            rowacc[:],
                            rowacc[:],
                            sgrp[:, g * R : (g + 1) * R],
                            op=mybir.AluOpType.min,
                        )
                    # batched colmin tree over the G blocks (bf16 2x TTs + one
                    # small 1x reduce; the accumulate-reduce opcode has no
                    # fast uop so a TT tree is cheaper)
                    sv = sgrp[:].rearrange("p (g n) -> p g n", g=G)
                    tr = trpool.tile([128, G * (R // 2)], s_dt, tag="tree")
                    tv = tr[:].rearrange("p (g n) -> p g n", g=G)
                    nc.vector.tensor_tensor(
                        tv[:, :, :], sv[:, :, 0 : R // 2], sv[:, :, R // 2 : R],
                        op=mybir.AluOpType.min,
                    )
                    w = R // 4
                    while w >= 64:
                        nc.vector.tensor_tensor(
                            tv[:, :, 0:w], tv[:, :, 0:w], tv[:, :, w : 2 * w],
                            op=mybir.AluOpType.min,
                        )
                        w //= 2
                    nc.vector.tensor_reduce(
                        out=colmin[:, tbg * G : (tbg + 1) * G],
                        in_=tv[:, :, 0:64],
                        axis=mybir.AxisListType.X,
                        op=mybir.AluOpType.min,
                    )

            # fold rowacc across partitions: transpose 128x128 blocks (fp32),
            # then min-reduce the free dim
            if EXACT:
                rowacc32 = rowacc
            else:
                nc.vector.tensor_copy(rowacc32[:], rowacc[:])
            for t in range(R // 128):
                tr_ps = pspool.tile([128, 128], F32, tag="s_ps")
                nc.tensor.transpose(
                    tr_ps[:], rowacc32[:, t * 128 : (t + 1) * 128], ident[:]
                )
                nc.vector.tensor_reduce(
                    out=orow[:, t : t + 1],
                    in_=tr_ps[:],
                    axis=mybir.AxisListType.X,
                    op=mybir.AluOpType.min,
                )

            nc.sync.dma_start(o_col_d.ap(), colmin[:])
            nc.sync.dma_start(o_row_d.ap(), orow[:])

    nc.compile()
    return nc


def _prepare_inputs(pred, target):
    pred = np.ascontiguousarray(np.asarray(pred, dtype=np.float32))
    target = np.ascontiguousarray(np.asarray(target, dtype=np.float32))
    t2 = (target * target).sum(axis=1)
    p2 = (pred * pred).sum(axis=1)
    ones_m = np.ones(M, dtype=np.float32)
    t_aug = np.stack(
        [-2.0 * target[:, 0], -2.0 * target[:, 1], -2.0 * target[:, 2], t2, ones_m],
        axis=0,
    ).astype(np.float32)
    t_aug = np.ascontiguousarray(t_aug)
    # first weight chunk pre-replicated at partition bases 0/32/64/96
    t_w0 = np.zeros((128, 1024), dtype=np.float32)
    for i in range(4):
        t_w0[32 * i : 32 * i + 5, :] = t_aug[:, 0:1024]
    t_w0 = np.ascontiguousarray(t_w0)
    in_maps = []
    for k in range(N_CORES):
        sl = slice(k * R, (k + 1) * R)
        p = pred[sl]
        p_aug5 = np.stack(
            [p[:, 0], p[:, 1], p[:, 2], np.ones(R, dtype=np.float32), p2[sl]], axis=0
        ).astype(np.float32)
        p_aug = np.zeros((128, R), dtype=np.float32)
        for i in range(4):
            p_aug[32 * i : 32 * i + 5, :] = p_aug5
        in_maps.append(
            {
                "t_aug": t_aug,
                "t_w0": t_w0,
                "p_aug": np.ascontiguousarray(p_aug),
            }
        )
    return in_maps


def _run(pred, target, trace=False):
    if "nc" not in _cache:
        _cache["nc"] = _build()
    nc = _cache["nc"]
    in_maps = _prepare_inputs(pred, target)
    res = run_bass_kernel_spmd(nc, in_maps, core_ids=list(range(N_CORES)), trace=trace)

    rowmins = []
    colparts = []
    for k in range(N_CORES):
        out = res.results[k]
        # o_row[q, t] -> pred local index t*128+q
        rowmins.append(out["o_row"].T.reshape(-1))
        # o_col[p, tb] -> target index tb*128+p
        colparts.append(out["o_col"].T.reshape(-1))
    rowmin_sq = np.concatenate(rowmins)  # [16384] squared forward mins
    colmin_sq = np.min(np.stack(colparts, axis=0), axis=0)  # [16384]

    fwd = np.sqrt(np.maximum(rowmin_sq, 0.0)).mean()
    bwd = np.sqrt(np.maximum(colmin_sq, 0.0)).mean()
    value = np.float32((fwd + bwd) / 2.0)
    return np.asarray(value, dtype=np.float32), res


def kernel(pred, target):
    out, _ = _run(pred, target, trace=False)
    return out


# revision 28
# speedup vs baseline: 1.2977x; 1.0115x over previous
"""Chamfer loss kernel for Trainium2 (8 NeuronCores).

Problem: pred [16384, 3], target [16384, 3] fp32.
  dist[i, j] = ||pred_i - target_j||
  out = (mean_i min_j dist + mean_j min_i dist) / 2

Strategy:
  - Shard pred rows across the 8 cores (2048 rows each); every core sees the
    full target set.
  - On each core compute the transposed squared-distance matrix
        S^T[t, p] = ||pred_p - target_t||^2
    via a single K=5 augmented matmul:
        lhsT rows = [-2*tx, -2*ty, -2*tz, |t|^2, 1]   (stationary, targets)
        rhs  rows = [ px,    py,   pz,   1,  |p|^2]   (moving, preds)
    so the TensorEngine emits squared distances directly into PSUM.
    Since K=5 only uses 5 of the 128 PE array rows, the 4 pred-chunks of a
    target block are packed into 4 concurrent row-group matmuls
    (tile_position=(32*i, 0)) writing 4 different PSUM banks.
  - VectorE per 128-target block:
      * tensor_scalar (min, +inf) reading PSUM fp32 with a free-dim
        min-accumulate -> exact backward-direction partial min over this
        core's 2048 preds; its elementwise output doubles as the SBUF
        staging copy (cast to bf16 unless CHAMFER_EXACT=1).
      * tensor_tensor min into a [128, 2048] running accumulator (bf16 runs
        in the DVE 2x mode) -> forward-direction partial.
  - Tail: 16 PE transposes + free-dim min reduces fold the forward
    accumulator across partitions.
  - Host: sqrt/clamp/means + 8-way elementwise min for the backward
    direction (min over squared distances commutes with the monotone
    sqrt(max(.,0))), so reducing over squared distances is exact.
"""

import os
import sys

if "/opt/trn_rl_repo" not in sys.path:
    sys.path.insert(0, "/opt/trn_rl_repo")

from contextlib import ExitStack

import numpy as np

import concourse.bass as bass
import concourse.mybir as mybir
import concourse.tile as tile
from concourse import bacc
from concourse.bass_utils import run_bass_kernel_spmd
from concourse.masks import make_identity

F32 = mybir.dt.float32
BF16 = mybir.dt.bfloat16
BIG = 3.0e38

N_CORES = 8
N = 16384  # pred rows (global)
M = 16384  # target rows
R = N // N_CORES  # pred rows per core = 2048
TB = M // 128  # target blocks = 128
PC = R // 512  # pred chunks per core = 4 (one per PE row-group)

EXACT = bool(int(os.environ.get("CHAMFER_EXACT", "0")))

_cache = {}


def _build():
    s_dt = F32 if EXACT else BF16

    nc = bacc.Bacc("TRN2", target_bir_lowering=False, debug=False, num_devices=N_CORES)

    t_aug_d = nc.dram_tensor("t_aug", [5, M], F32, kind="ExternalInput")
    # rhs and first weight chunk come pre-replicated from the host as dense
    # 128-partition tensors -> one fast full-width DMA each instead of many
    # few-partition transfers on the critical path
    p_aug_d = nc.dram_tensor("p_aug", [128, R], F32, kind="ExternalInput")
    t_w0_d = nc.dram_tensor("t_w0", [128, 1024], F32, kind="ExternalInput")
    o_col_d = nc.dram_tensor("o_col", [128, TB], F32, kind="ExternalOutput")
    o_row_d = nc.dram_tensor("o_row", [128, R // 128], F32, kind="ExternalOutput")

    with tile.TileContext(nc) as tc:
        with ExitStack() as ctx:
            const = ctx.enter_context(tc.tile_pool(name="const", bufs=1))
            spool = ctx.enter_context(tc.tile_pool(name="spool", bufs=2))
            trpool = ctx.enter_context(tc.tile_pool(name="trpool", bufs=1))
            pspool = ctx.enter_context(tc.tile_pool(name="pspool", bufs=2, space="PSUM"))

            # weights replicated at partition bases 0/32/64/96 for the
            # 4 concurrent row-group matmuls; loads spread over all three DMA
            # issuers (sync/act HWDGE + gpsimd SWDGE) and column-chunked so
            # compute starts while the rest of the weights stream in behind
            t_w = const.tile([128, M], F32)
            p_r = const.tile([128, R], F32)
            nc.sync.dma_start(p_r[:], p_aug_d.ap())
            nc.scalar.dma_start(t_w[:, 0:1024], t_w0_d.ap())
            dma_engines = [nc.gpsimd, nc.sync, nc.scalar]
            idx = 0
            col_chunks = [512] * 2 + [2048] * 7
            c0 = 1024
            for ch in col_chunks:
                cols = slice(c0, c0 + ch)
                c0 += ch
                for i in range(4):
                    dma_engines[idx % 3].dma_start(
                        t_w[32 * i : 32 * i + 5, cols], t_aug_d.ap()[:, cols]
                    )
                    idx += 1

            ident = const.tile([128, 128], F32)
            make_identity(nc, ident[:])

            rowacc = const.tile([128, R], s_dt)
            nc.vector.memset(rowacc[:], BIG)
            rowacc32 = const.tile([128, R], F32)
            colmin = const.tile([128, TB], F32)
            orow = const.tile([128, R // 128], F32)

            if EXACT:
                for tb in range(TB):
                    s_ps = pspool.tile([128, R], F32, tag="s_ps")
                    for pc in range(PC):
                        nc.tensor.matmul(
                            s_ps[:, pc * 512 : (pc + 1) * 512],
                            t_w[32 * pc : 32 * pc + 5, tb * 128 : (tb + 1) * 128],
                            p_r[32 * pc : 32 * pc + 5, pc * 512 : (pc + 1) * 512],
                            start=True,
                            stop=True,
                            tile_position=(32 * pc, 0),
                        )
                    s_sb = spool.tile([128, R], s_dt, tag="s_sb")
                    # backward partial: exact fp32 min over this core's preds
                    # for each target; elementwise out is the SBUF staging copy
                    nc.vector.tensor_scalar(
                        out=s_sb[:],
                        in0=s_ps[:],
                        scalar1=BIG,
                        scalar2=None,
                        op0=mybir.AluOpType.min,
                        op1=mybir.AluOpType.min,
                        accum_out=colmin[:, tb : tb + 1],
                    )
                    # forward running min across target blocks
                    nc.vector.tensor_tensor(
                        rowacc[:], rowacc[:], s_sb[:], op=mybir.AluOpType.min
                    )
            else:
                G = 8  # target blocks per colmin-tree batch
                for tbg in range(TB // G):
                    sgrp = spool.tile([128, G * R], s_dt, tag="s_sb")
                    tr = trpool.tile([128, G * (R // 2)], s_dt, tag="tree")
                    for g in range(G):
                        tb = tbg * G + g
                        s_ps = pspool.tile([128, R], F32, tag="s_ps")
                        for pc in range(PC):
                            nc.tensor.matmul(
                                s_ps[:, pc * 512 : (pc + 1) * 512],
                                t_w[32 * pc : 32 * pc + 5, tb * 128 : (tb + 1) * 128],
                                p_r[32 * pc : 32 * pc + 5, pc * 512 : (pc + 1) * 512],
                                start=True,
                                stop=True,
                                tile_position=(32 * pc, 0),
                            )
                        # ScalarE does the PSUM->SBUF bf16 cast
                        nc.scalar.copy(sgrp[:, g * R : (g + 1) * R], s_ps[:])
                        # forward running min across target blocks (bf16 2x)
                        nc.vector.tensor_tensor(
                            rowacc[:],
                            rowacc[:],
                            sgrp[:, g * R : (g + 1) * R],
                            op=mybir.AluOpType.min,
                        )
                        # colmin tree level 1 per block: contiguous 2-D ops
                        # run closer to the DVE cost model than one big
                        # strided 3-D op
                        nc.vector.tensor_tensor(
                            tr[:, g * (R // 2) : (g + 1) * (R // 2)],
                            sgrp[:, g * R : g * R + R // 2],
                            sgrp[:, g * R + R // 2 : (g + 1) * R],
                            op=mybir.AluOpType.min,
                        )
                    # batched colmin tree levels 2+ over the G blocks (bf16 2x
                    # TTs + one small 1x reduce; the accumulate-reduce opcode
                    # has no fast uop so a TT tree is cheaper)
                    tv = tr[:].rearrange("p (g n) -> p g n", g=G)
                    w = R // 4
                    while w >= 64:
                        nc.vector.tensor_tensor(
                            tv[:, :, 0:w], tv[:, :, 0:w], tv[:, :, w : 2 * w],
                            op=mybir.AluOpType.min,
                        )
                        w //= 2
                    nc.vector.tensor_reduce(
                        out=colmin[:, tbg * G : (tbg + 1) * G],
                        in_=tv[:, :, 0:64],
                        axis=mybir.AxisListType.X,
                        op=mybir.AluOpType.min,
                    )

            # fold rowacc across partitions: transpose 128x128 blocks (fp32),
            # then min-reduce the free dim
            if EXACT:
                rowacc32 = rowacc
            else:
                nc.vector.tensor_copy(rowacc32[:], rowacc[:])
            for t in range(R // 128):
                tr_ps = pspool.tile([128, 128], F32, tag="s_ps")
                nc.tensor.transpose(
                    tr_ps[:], rowacc32[:, t * 128 : (t + 1) * 128], ident[:]
                )
                nc.vector.tensor_reduce(
                    out=orow[:, t : t + 1],
                    in_=tr_ps[:],
                    axis=mybir.AxisListType.X,
                    op=mybir.AluOpType.min,
                )

            nc.sync.dma_start(o_col_d.ap(), colmin[:])
            nc.sync.dma_start(o_row_d.ap(), orow[:])

    nc.compile()
    return nc


def _prepare_inputs(pred, target):
    pred = np.ascontiguousarray(np.asarray(pred, dtype=np.float32))
    target = np.ascontiguousarray(np.asarray(target, dtype=np.float32))
    t2 = (target * target).sum(axis=1)
    p2 = (pred * pred).sum(axis=1)
    ones_m = np.ones(M, dtype=np.float32)
    t_aug = np.stack(
        [-2.0 * target[:, 0], -2.0 * target[:, 1], -2.0 * target[:, 2], t2, ones_m],
        axis=0,
    ).astype(np.float32)
    t_aug = np.ascontiguousarray(t_aug)
    # first weight chunk pre-replicated at partition bases 0/32/64/96
    t_w0 = np.zeros((128, 1024), dtype=np.float32)
    for i in range(4):
        t_w0[32 * i : 32 * i + 5, :] = t_aug[:, 0:1024]
    t_w0 = np.ascontiguousarray(t_w0)
    in_maps = []
    for k in range(N_CORES):
        sl = slice(k * R, (k + 1) * R)
        p = pred[sl]
        p_aug5 = np.stack(
            [p[:, 0], p[:, 1], p[:, 2], np.ones(R, dtype=np.float32), p2[sl]], axis=0
        ).astype(np.float32)
        p_aug = np.zeros((128, R), dtype=np.float32)
        for i in range(4):
            p_aug[32 * i : 32 * i + 5, :] = p_aug5
        in_maps.append(
            {
                "t_aug": t_aug,
                "t_w0": t_w0,
                "p_aug": np.ascontiguousarray(p_aug),
            }
        )
    return in_maps


def _run(pred, target, trace=False):
    if "nc" not in _cache:
        _cache["nc"] = _build()
    nc = _cache["nc"]
    in_maps = _prepare_inputs(pred, target)
    res = run_bass_kernel_spmd(nc, in_maps, core_ids=list(range(N_CORES)), trace=trace)

    rowmins = []
    colparts = []
    for k in range(N_CORES):
        out = res.results[k]
        # o_row[q, t] -> pred local index t*128+q
        rowmins.append(out["o_row"].T.reshape(-1))
        # o_col[p, tb] -> target index tb*128+p
        colparts.append(out["o_col"].T.reshape(-1))
    rowmin_sq = np.concatenate(rowmins)  # [16384] squared forward mins
    colmin_sq = np.min(np.stack(colparts, axis=0), axis=0)  # [16384]

    fwd = np.sqrt(np.maximum(rowmin_sq, 0.0)).mean()
    bwd = np.sqrt(np.maximum(colmin_sq, 0.0)).mean()
    value = np.float32((fwd + bwd) / 2.0)
    return np.asarray(value, dtype=np.float32), res


def kernel(pred, target):
    out, _ = _run(pred, target, trace=False)
    return out


# revision 31
# speedup vs baseline: 1.2982x; 1.0004x over previous
"""Chamfer loss kernel for Trainium2 (8 NeuronCores).

Problem: pred [16384, 3], target [16384, 3] fp32.
  dist[i, j] = ||pred_i - target_j||
  out = (mean_i min_j dist + mean_j min_i dist) / 2

Strategy:
  - Shard pred rows across the 8 cores (2048 rows each); every core sees the
    full target set.
  - On each core compute the transposed squared-distance matrix
        S^T[t, p] = ||pred_p - target_t||^2
    via a single K=5 augmented matmul:
        lhsT rows = [-2*tx, -2*ty, -2*tz, |t|^2, 1]   (stationary, targets)
        rhs  rows = [ px,    py,   pz,   1,  |p|^2]   (moving, preds)
    so the TensorEngine emits squared distances directly into PSUM.
    Since K=5 only uses 5 of the 128 PE array rows, the 4 pred-chunks of a
    target block are packed into 4 concurrent row-group matmuls
    (tile_position=(32*i, 0)) writing 4 different PSUM banks.
  - VectorE per 128-target block:
      * tensor_scalar (min, +inf) reading PSUM fp32 with a free-dim
        min-accumulate -> exact backward-direction partial min over this
        core's 2048 preds; its elementwise output doubles as the SBUF
        staging copy (cast to bf16 unless CHAMFER_EXACT=1).
      * tensor_tensor min into a [128, 2048] running accumulator (bf16 runs
        in the DVE 2x mode) -> forward-direction partial.
  - Tail: 16 PE transposes + free-dim min reduces fold the forward
    accumulator across partitions.
  - Host: sqrt/clamp/means + 8-way elementwise min for the backward
    direction (min over squared distances commutes with the monotone
    sqrt(max(.,0))), so reducing over squared distances is exact.
"""

import os
import sys

if "/opt/trn_rl_repo" not in sys.path:
    sys.path.insert(0, "/opt/trn_rl_repo")

from contextlib import ExitStack

import numpy as np

import concourse.bass as bass
import concourse.mybir as mybir
import concourse.tile as tile
from concourse import bacc
from concourse.bass_utils import run_bass_kernel_spmd
from concourse.masks import make_identity

F32 = mybir.dt.float32
BF16 = mybir.dt.bfloat16
BIG = 3.0e38

N_CORES = 8
N = 16384  # pred rows (global)
M = 16384  # target rows
R = N // N_CORES  # pred rows per core = 2048
TB = M // 128  # target blocks = 128
PC = R // 512  # pred chunks per core = 4 (one per PE row-group)

EXACT = bool(int(os.environ.get("CHAMFER_EXACT", "0")))

_cache = {}


def _build():
    s_dt = F32 if EXACT else BF16

    nc = bacc.Bacc("TRN2", target_bir_lowering=False, debug=False, num_devices=N_CORES)

    t_aug_d = nc.dram_tensor("t_aug", [5, M], F32, kind="ExternalInput")
    # rhs and first weight chunk come pre-replicated from the host as dense
    # 128-partition tensors -> one fast full-width DMA each instead of many
    # few-partition transfers on the critical path
    p_aug_d = nc.dram_tensor("p_aug", [128, R], F32, kind="ExternalInput")
    t_w0_d = nc.dram_tensor("t_w0", [128, 1024], F32, kind="ExternalInput")
    o_col_d = nc.dram_tensor("o_col", [128, TB], F32, kind="ExternalOutput")
    o_row_d = nc.dram_tensor("o_row", [128, R // 128], F32, kind="ExternalOutput")

    with tile.TileContext(nc) as tc:
        with ExitStack() as ctx:
            const = ctx.enter_context(tc.tile_pool(name="const", bufs=1))
            spool = ctx.enter_context(tc.tile_pool(name="spool", bufs=2))
            trpool = ctx.enter_context(tc.tile_pool(name="trpool", bufs=1))
            pspool = ctx.enter_context(tc.tile_pool(name="pspool", bufs=2, space="PSUM"))

            # weights replicated at partition bases 0/32/64/96 for the
            # 4 concurrent row-group matmuls; loads spread over all three DMA
            # issuers (sync/act HWDGE + gpsimd SWDGE) and column-chunked so
            # compute starts while the rest of the weights stream in behind
            t_w = const.tile([128, M], F32)
            p_r = const.tile([128, R], F32)
            nc.sync.dma_start(p_r[:], p_aug_d.ap())
            nc.scalar.dma_start(t_w[:, 0:1024], t_w0_d.ap())
            dma_engines = [nc.gpsimd, nc.sync, nc.scalar]
            idx = 0
            col_chunks = [512] * 2 + [2048] * 7
            c0 = 1024
            for ch in col_chunks:
                cols = slice(c0, c0 + ch)
                c0 += ch
                for i in range(4):
                    dma_engines[idx % len(dma_engines)].dma_start(
                        t_w[32 * i : 32 * i + 5, cols], t_aug_d.ap()[:, cols]
                    )
                    idx += 1

            ident = const.tile([128, 128], F32)
            make_identity(nc, ident[:])

            rowacc = const.tile([128, R], s_dt)
            nc.vector.memset(rowacc[:], BIG)
            rowacc32 = const.tile([128, R], F32)
            colmin = const.tile([128, TB], F32)
            orow = const.tile([128, R // 128], F32)

            if EXACT:
                for tb in range(TB):
                    s_ps = pspool.tile([128, R], F32, tag="s_ps")
                    for pc in range(PC):
                        nc.tensor.matmul(
                            s_ps[:, pc * 512 : (pc + 1) * 512],
                            t_w[32 * pc : 32 * pc + 5, tb * 128 : (tb + 1) * 128],
                            p_r[32 * pc : 32 * pc + 5, pc * 512 : (pc + 1) * 512],
                            start=True,
                            stop=True,
                            tile_position=(32 * pc, 0),
                        )
                    s_sb = spool.tile([128, R], s_dt, tag="s_sb")
                    # backward partial: exact fp32 min over this core's preds
                    # for each target; elementwise out is the SBUF staging copy
                    nc.vector.tensor_scalar(
                        out=s_sb[:],
                        in0=s_ps[:],
                        scalar1=BIG,
                        scalar2=None,
                        op0=mybir.AluOpType.min,
                        op1=mybir.AluOpType.min,
                        accum_out=colmin[:, tb : tb + 1],
                    )
                    # forward running min across target blocks
                    nc.vector.tensor_tensor(
                        rowacc[:], rowacc[:], s_sb[:], op=mybir.AluOpType.min
                    )
            else:
                G = 8  # target blocks per colmin-tree batch
                for tbg in range(TB // G):
                    sgrp = spool.tile([128, G * R], s_dt, tag="s_sb")
                    tr = trpool.tile([128, G * (R // 2)], s_dt, tag="tree")
                    for g in range(G):
                        tb = tbg * G + g
                        s_ps = pspool.tile([128, R], F32, tag="s_ps")
                        for pc in range(PC):
                            nc.tensor.matmul(
                                s_ps[:, pc * 512 : (pc + 1) * 512],
                                t_w[32 * pc : 32 * pc + 5, tb * 128 : (tb + 1) * 128],
                                p_r[32 * pc : 32 * pc + 5, pc * 512 : (pc + 1) * 512],
                                start=True,
                                stop=True,
                                tile_position=(32 * pc, 0),
                            )
                        # ScalarE does the PSUM->SBUF bf16 cast
                        nc.scalar.copy(sgrp[:, g * R : (g + 1) * R], s_ps[:])
                        # forward running min across target blocks (bf16 2x)
                        nc.vector.tensor_tensor(
                            rowacc[:],
                            rowacc[:],
                            sgrp[:, g * R : (g + 1) * R],
                            op=mybir.AluOpType.min,
                        )
                        # colmin tree level 1 per block: contiguous 2-D ops
                        # run closer to the DVE cost model than one big
                        # strided 3-D op
                        nc.vector.tensor_tensor(
                            tr[:, g * (R // 2) : (g + 1) * (R // 2)],
                            sgrp[:, g * R : g * R + R // 2],
                            sgrp[:, g * R + R // 2 : (g + 1) * R],
                            op=mybir.AluOpType.min,
                        )
                    # batched colmin tree levels 2+ over the G blocks (bf16 2x
                    # TTs + one small 1x reduce; the accumulate-reduce opcode
                    # has no fast uop so a TT tree is cheaper)
                    tv = tr[:].rearrange("p (g n) -> p g n", g=G)
                    w = R // 4
                    while w >= 64:
                        nc.vector.tensor_tensor(
                            tv[:, :, 0:w], tv[:, :, 0:w], tv[:, :, w : 2 * w],
                            op=mybir.AluOpType.min,
                        )
                        w //= 2
                    nc.vector.tensor_reduce(
                        out=colmin[:, tbg * G : (tbg + 1) * G],
                        in_=tv[:, :, 0:64],
                        axis=mybir.AxisListType.X,
                        op=mybir.AluOpType.min,
                    )

            # fold rowacc across partitions: transpose 128x128 blocks (fp32),
            # then min-reduce the free dim
            if EXACT:
                rowacc32 = rowacc
            else:
                nc.vector.tensor_copy(rowacc32[:], rowacc[:])
            for t in range(R // 128):
                tr_ps = pspool.tile([128, 128], F32, tag="s_ps")
                nc.tensor.transpose(
                    tr_ps[:], rowacc32[:, t * 128 : (t + 1) * 128], ident[:]
                )
                nc.vector.tensor_reduce(
                    out=orow[:, t : t + 1],
                    in_=tr_ps[:],
                    axis=mybir.AxisListType.X,
                    op=mybir.AluOpType.min,
                )

            nc.sync.dma_start(o_col_d.ap(), colmin[:])
            nc.sync.dma_start(o_row_d.ap(), orow[:])

    nc.compile()
    return nc


def _prepare_inputs(pred, target):
    pred = np.ascontiguousarray(np.asarray(pred, dtype=np.float32))
    target = np.ascontiguousarray(np.asarray(target, dtype=np.float32))
    t2 = (target * target).sum(axis=1)
    p2 = (pred * pred).sum(axis=1)
    ones_m = np.ones(M, dtype=np.float32)
    t_aug = np.stack(
        [-2.0 * target[:, 0], -2.0 * target[:, 1], -2.0 * target[:, 2], t2, ones_m],
        axis=0,
    ).astype(np.float32)
    t_aug = np.ascontiguousarray(t_aug)
    # first weight chunk pre-replicated at partition bases 0/32/64/96
    t_w0 = np.zeros((128, 1024), dtype=np.float32)
    for i in range(4):
        t_w0[32 * i : 32 * i + 5, :] = t_aug[:, 0:1024]
    t_w0 = np.ascontiguousarray(t_w0)
    in_maps = []
    for k in range(N_CORES):
        sl = slice(k * R, (k + 1) * R)
        p = pred[sl]
        p_aug5 = np.stack(
            [p[:, 0], p[:, 1], p[:, 2], np.ones(R, dtype=np.float32), p2[sl]], axis=0
        ).astype(np.float32)
        p_aug = np.zeros((128, R), dtype=np.float32)
        for i in range(4):
            p_aug[32 * i : 32 * i + 5, :] = p_aug5
        in_maps.append(
            {
                "t_aug": t_aug,
                "t_w0": t_w0,
                "p_aug": np.ascontiguousarray(p_aug),
            }
        )
    return in_maps


def _run(pred, target, trace=False):
    if "nc" not in _cache:
        _cache["nc"] = _build()
    nc = _cache["nc"]
    in_maps = _prepare_inputs(pred, target)
    res = run_bass_kernel_spmd(nc, in_maps, core_ids=list(range(N_CORES)), trace=trace)

    rowmins = []
    colparts = []
    for k in range(N_CORES):
        out = res.results[k]
        # o_row[q, t] -> pred local index t*128+q
        rowmins.append(out["o_row"].T.reshape(-1))
        # o_col[p, tb] -> target index tb*128+p
        colparts.append(out["o_col"].T.reshape(-1))
    rowmin_sq = np.concatenate(rowmins)  # [16384] squared forward mins
    colmin_sq = np.min(np.stack(colparts, axis=0), axis=0)  # [16384]

    fwd = np.sqrt(np.maximum(rowmin_sq, 0.0)).mean()
    bwd = np.sqrt(np.maximum(colmin_sq, 0.0)).mean()
    value = np.float32((fwd + bwd) / 2.0)
    return np.asarray(value, dtype=np.float32), res


def kernel(pred, target):
    out, _ = _run(pred, target, trace=False)
    return out
